# revision 22
# baseline (speedup 1.0000x reference)
"""Trainium2 Bass kernel for iterative Hopfield associative memory retrieval.

Reference computation (per problem):
    for 3 steps: scores = q @ K^T * beta; attn = softmax(scores); q = attn @ K
    retrieved = attn @ V
    returns (retrieved, attn)

Sharding: data-parallel over the flattened (batch*token) dim across 8 cores;
keys/values replicated. Each core handles 2048 tokens.

Per-core kernel strategy (v2):
  - Scores are computed transposed (S^T [k, t]) so exp(S^T) tiles have the
    contraction dim k on partitions and feed the q-update / retrieval matmuls
    as stationary operands directly.
  - K and V are augmented with a ones column (K_aug [k, 257]) used as the
    moving operand: each accumulation produces the updated q (natural layout)
    PLUS the softmax row-sum in column 256 — no separate row-sum matmuls.
  - q-update: normalize by 1/rowsum (per-partition scalar), transpose back to
    [d, t] for the next step's scores.
  - Final step: the augmented-V accumulation yields retrieved (natural) +
    rowsums; attention is recomputed in natural [t, k] orientation with
    exp(beta*s - ln(rowsum)) fused in one ACT pass and DMA'd out directly.
  - All matmuls run float32r (full-rate fp32 streaming), fp32 PSUM accum.
  - Emission is software-pipelined: scores(kc+1) is emitted before the
    q-update of kc; per-tile epilogues are emitted after the next tile's
    main loop so the PE never waits on the normalize chain.
"""

from contextlib import ExitStack

import numpy as np

import concourse.bass as bass
import concourse.tile as tile
from concourse import bacc, mybir
from concourse.bass_utils import run_bass_kernel_spmd
from concourse.masks import make_identity

N_CORES = 8
B, TFULL, D = 2, 8192, 256
K = 4096
T = B * TFULL // N_CORES  # tokens per core = 2048
P = 128
DA = D + 2  # ones column (-> rowsum) + pad to even (fp32r ISA rule)
TT = 512  # token tile
KC = K // P  # 32 key chunks
DC = D // P  # 2 dim chunks
NSTEPS = 3

F32 = mybir.dt.float32
F32R = mybir.dt.float32r
EXP = mybir.ActivationFunctionType.Exp
LOG = mybir.ActivationFunctionType.Ln


def build_kernel(beta: float, n_tok: int = T, reps: int = 1):
    t_tiles = max(1, n_tok // TT)
    tt_sz = min(TT, n_tok)
    n_sub = tt_sz // P
    n_tc = n_tok // P
    nc = bacc.Bacc("TRN2", target_bir_lowering=False, debug=False,
                   num_devices=N_CORES)
    q_dram = nc.dram_tensor("query", [n_tok, D], F32, kind="ExternalInput").ap()
    k_dram = nc.dram_tensor("keys", [K, D], F32, kind="ExternalInput").ap()
    v_dram = nc.dram_tensor("values", [K, D], F32, kind="ExternalInput").ap()
    retr_dram = nc.dram_tensor("retrieved", [n_tok, D], F32,
                               kind="ExternalOutput").ap()
    attn_dram = nc.dram_tensor("attn", [n_tok, K], F32,
                               kind="ExternalOutput").ap()

    with tile.TileContext(nc) as tc, ExitStack() as ctx, \
            nc.allow_low_precision(reason="fp32r rounding is intended"):
        consts = ctx.enter_context(tc.tile_pool(name="consts", bufs=1))
        big = ctx.enter_context(tc.tile_pool(name="big", bufs=1))
        raw = ctx.enter_context(tc.tile_pool(name="raw", bufs=1))
        qt_pool = ctx.enter_context(tc.tile_pool(name="qtp", bufs=1))
        et_pool = ctx.enter_context(tc.tile_pool(name="etp", bufs=4))
        small = ctx.enter_context(tc.tile_pool(name="small", bufs=4))
        ostage = ctx.enter_context(tc.tile_pool(name="ostage", bufs=4))
        ps_st = ctx.enter_context(tc.tile_pool(name="ps_st", bufs=3,
                                               space="PSUM"))
        ps_acc = ctx.enter_context(tc.tile_pool(name="ps_acc", bufs=1,
                                                space="PSUM"))

        ident = consts.tile([P, P], F32)
        make_identity(nc, ident)

        # K_aug / V_aug: [p, kc, 0:256] = row kc*P+p; col 256 = 1.0
        k_aug = big.tile([P, KC, DA], F32R)
        v_aug = big.tile([P, KC, DA], F32R)
        kt_sb = big.tile([P, DC, K], F32R)

        def load_aug(mat_dram, mat_aug):
            stage = raw.tile([P, KC, DA], F32, tag="raw")
            nc.sync.dma_start(stage[:, :, 0:D],
                              mat_dram.rearrange("(kc p) d -> p kc d", p=P))
            nc.vector.memset(stage[:, :, D:DA], 1.0)
            nc.vector.tensor_copy(mat_aug[:], stage[:])

        CH = 4  # k-chunks per load chunk
        for c0 in range(0, KC, CH):
            stage_c = raw.tile([P, CH, DA], F32, tag="rawc", bufs=2)
            nc.sync.dma_start(
                stage_c[:, :, 0:D],
                k_dram[c0 * P:(c0 + CH) * P, :].rearrange(
                    "(kc p) d -> p kc d", p=P))
            nc.vector.memset(stage_c[:, :, D:DA], 1.0)
            nc.vector.tensor_copy(k_aug[:, c0:c0 + CH, :], stage_c[:])
            for kc in range(c0, c0 + CH):
                for dc in range(DC):
                    tp = ps_st.tile([P, P], F32, tag="st")
                    nc.tensor.transpose(
                        tp, k_aug[:, kc, dc * P:(dc + 1) * P].bitcast(F32),
                        ident)
                    nc.scalar.copy(kt_sb[:, dc, kc * P:(kc + 1) * P], tp)

        first_rep = True
        for _ in range(reps):
            # Q^T: qt[p, dc, t] = q[t, dc*P+p]; updated in place each step.
            q_raw = raw.tile([P, n_tc, D], F32, tag="raw")
            nc.sync.dma_start(q_raw[:],
                              q_dram.rearrange("(tc p) d -> p tc d", p=P))
            qts = [qt_pool.tile([P, DC, tt_sz], F32R, tag=f"qt{tt}",
                                name=f"qt{tt}")
                   for tt in range(t_tiles)]
            for tcn in range(n_tc):
                tt, off = divmod(tcn * P, tt_sz)
                for dc in range(DC):
                    tp = ps_st.tile([P, P], F32, tag="st")
                    nc.tensor.transpose(tp, q_raw[:, tcn, dc * P:(dc + 1) * P],
                                        ident)
                    nc.scalar.copy(qts[tt][:, dc, off:off + P], tp)
            if first_rep:
                first_rep = False
                load_aug(v_dram, v_aug)  # needed only in the last step

            pending = None  # lazily emitted epilogue of previous t-tile
            for step in range(NSTEPS):
                last = step == NSTEPS - 1
                mat = v_aug if last else k_aug

                for tt in range(t_tiles):
                    if pending is not None and pending[1] == tt:
                        # epilogue writes qt[tt]; must precede this tile's
                        # scores in program order
                        pending[0]()
                        pending = None
                    qt = qts[tt]
                    acc = ps_acc.tile([P, n_sub, DA], F32, tag="acc",
                                      padded_shape=[P, n_sub, 512])

                    def qupd(pet, pkc, acc=acc, mat=mat):
                        for j in range(n_sub):
                            nc.tensor.matmul(
                                acc[:, j, :],
                                pet[:, bass.ts(j, P)],
                                mat[:, pkc, :],
                                start=(pkc == 0), stop=(pkc == KC - 1))

                    prev = None
                    for kc in range(KC):
                        st = ps_st.tile([P, tt_sz], F32, tag="st")
                        for dc in range(DC):
                            nc.tensor.matmul(
                                st,
                                kt_sb[:, dc, kc * P:(kc + 1) * P],
                                qt[:, dc, :],
                                start=(dc == 0), stop=(dc == DC - 1))
                        et = et_pool.tile([P, tt_sz], F32R, tag="et", bufs=6)
                        nc.scalar.activation(et, st, EXP, scale=beta)
                        if prev is not None:
                            qupd(*prev)
                        prev = (et, kc)
                        if kc == 1 and pending is not None:
                            # previous tile's epilogue hides under this loop
                            pending[0]()
                            pending = None
                    qupd(*prev)

                    def epilogue(acc=acc, tt=tt, qt=qts[tt], last=last):
                        # single fast ACT copy releases the PSUM accumulator
                        acc_sb = small.tile([P, n_sub, DA], F32, tag="accs",
                                            bufs=2)
                        nc.vector.tensor_copy(acc_sb, acc)
                        for j in range(n_sub):
                            row0 = tt * tt_sz + j * P
                            rcp = small.tile([P, 1], F32, tag="rcp")
                            nc.vector.reciprocal(rcp, acc_sb[:, j, D:D + 1])
                            if not last:
                                qn = small.tile([P, D], F32, tag="qn")
                                nc.vector.tensor_scalar_mul(
                                    qn, acc_sb[:, j, 0:D], rcp)
                                tp = ps_st.tile([P, D], F32, tag="st")
                                for dc in range(DC):
                                    nc.tensor.transpose(
                                        tp[:, dc * P:(dc + 1) * P],
                                        qn[:, dc * P:(dc + 1) * P], ident)
                                for dc in range(DC):
                                    nc.vector.tensor_copy(
                                        qt[:, dc, j * P:(j + 1) * P],
                                        tp[:, dc * P:(dc + 1) * P])
                            else:
                                ret_sb = ostage.tile([P, D], F32, tag="ret")
                                nc.vector.tensor_scalar_mul(
                                    ret_sb, acc_sb[:, j, 0:D], rcp)
                                nc.sync.dma_start(
                                    retr_dram[row0:row0 + P, :], ret_sb)
                                # attention rows, natural orientation
                                for kb in range(K // 512):
                                    s3n = ps_st.tile([P, 512], F32, tag="st")
                                    for dc in range(DC):
                                        nc.tensor.matmul(
                                            s3n,
                                            qt[:, dc, j * P:(j + 1) * P],
                                            kt_sb[:, dc,
                                                  kb * 512:(kb + 1) * 512],
                                            start=(dc == 0),
                                            stop=(dc == DC - 1))
                                    a_sb = ostage.tile([P, 512], F32,
                                                       tag="attn", bufs=6)
                                    nc.scalar.activation(s3n, s3n, EXP,
                                                         scale=beta)
                                    nc.vector.tensor_scalar_mul(a_sb, s3n,
                                                                rcp)
                                    nc.sync.dma_start(
                                        attn_dram[row0:row0 + P,
                                                  kb * 512:(kb + 1) * 512],
                                        a_sb)

                    pending = (epilogue, tt)
            if pending is not None:
                pending[0]()

    nc.compile()
    return nc


def kernel(query, keys, values, log_beta):
    beta = float(np.exp(np.float32(log_beta)))
    nc = build_kernel(beta)
    qf = np.ascontiguousarray(
        np.asarray(query, dtype=np.float32).reshape(B * TFULL, D))
    keys = np.ascontiguousarray(np.asarray(keys, dtype=np.float32))
    values = np.ascontiguousarray(np.asarray(values, dtype=np.float32))
    in_maps = [
        {"query": qf[c * T:(c + 1) * T], "keys": keys, "values": values}
        for c in range(N_CORES)
    ]
    res = run_bass_kernel_spmd(nc, in_maps, core_ids=list(range(N_CORES)))
    retrieved = np.concatenate(
        [res.results[c]["retrieved"] for c in range(N_CORES)], axis=0)
    attn = np.concatenate(
        [res.results[c]["attn"] for c in range(N_CORES)], axis=0)
    return (retrieved.reshape(B, TFULL, D).astype(np.float32),
            attn.reshape(B, TFULL, K).astype(np.float32))


# revision 23
# speedup vs baseline: 1.0027x; 1.0027x over previous
"""Trainium2 Bass kernel for iterative Hopfield associative memory retrieval.

Reference computation (per problem):
    for 3 steps: scores = q @ K^T * beta; attn = softmax(scores); q = attn @ K
    retrieved = attn @ V
    returns (retrieved, attn)

Sharding: data-parallel over the flattened (batch*token) dim across 8 cores;
keys/values replicated. Each core handles 2048 tokens.

Per-core kernel strategy (v2):
  - Scores are computed transposed (S^T [k, t]) so exp(S^T) tiles have the
    contraction dim k on partitions and feed the q-update / retrieval matmuls
    as stationary operands directly.
  - K and V are augmented with a ones column (K_aug [k, 257]) used as the
    moving operand: each accumulation produces the updated q (natural layout)
    PLUS the softmax row-sum in column 256 — no separate row-sum matmuls.
  - q-update: normalize by 1/rowsum (per-partition scalar), transpose back to
    [d, t] for the next step's scores.
  - Final step: the augmented-V accumulation yields retrieved (natural) +
    rowsums; attention is recomputed in natural [t, k] orientation with
    exp(beta*s - ln(rowsum)) fused in one ACT pass and DMA'd out directly.
  - All matmuls run float32r (full-rate fp32 streaming), fp32 PSUM accum.
  - Emission is software-pipelined: scores(kc+1) is emitted before the
    q-update of kc; per-tile epilogues are emitted after the next tile's
    main loop so the PE never waits on the normalize chain.
"""

from contextlib import ExitStack

import numpy as np

import concourse.bass as bass
import concourse.tile as tile
from concourse import bacc, mybir
from concourse.bass_utils import run_bass_kernel_spmd
from concourse.masks import make_identity

N_CORES = 8
B, TFULL, D = 2, 8192, 256
K = 4096
T = B * TFULL // N_CORES  # tokens per core = 2048
P = 128
DA = D + 2  # ones column (-> rowsum) + pad to even (fp32r ISA rule)
TT = 512  # token tile
KC = K // P  # 32 key chunks
DC = D // P  # 2 dim chunks
NSTEPS = 3

F32 = mybir.dt.float32
F32R = mybir.dt.float32r
EXP = mybir.ActivationFunctionType.Exp
LOG = mybir.ActivationFunctionType.Ln


def build_kernel(beta: float, n_tok: int = T, reps: int = 1):
    t_tiles = max(1, n_tok // TT)
    tt_sz = min(TT, n_tok)
    n_sub = tt_sz // P
    n_tc = n_tok // P
    nc = bacc.Bacc("TRN2", target_bir_lowering=False, debug=False,
                   num_devices=N_CORES)
    q_dram = nc.dram_tensor("query", [n_tok, D], F32, kind="ExternalInput").ap()
    k_dram = nc.dram_tensor("keys", [K, D], F32, kind="ExternalInput").ap()
    v_dram = nc.dram_tensor("values", [K, D], F32, kind="ExternalInput").ap()
    retr_dram = nc.dram_tensor("retrieved", [n_tok, D], F32,
                               kind="ExternalOutput").ap()
    attn_dram = nc.dram_tensor("attn", [n_tok, K], F32,
                               kind="ExternalOutput").ap()

    with tile.TileContext(nc) as tc, ExitStack() as ctx, \
            nc.allow_low_precision(reason="fp32r rounding is intended"):
        consts = ctx.enter_context(tc.tile_pool(name="consts", bufs=1))
        big = ctx.enter_context(tc.tile_pool(name="big", bufs=1))
        raw = ctx.enter_context(tc.tile_pool(name="raw", bufs=1))
        qt_pool = ctx.enter_context(tc.tile_pool(name="qtp", bufs=1))
        et_pool = ctx.enter_context(tc.tile_pool(name="etp", bufs=4))
        small = ctx.enter_context(tc.tile_pool(name="small", bufs=4))
        ostage = ctx.enter_context(tc.tile_pool(name="ostage", bufs=4))
        ps_st = ctx.enter_context(tc.tile_pool(name="ps_st", bufs=3,
                                               space="PSUM"))
        ps_acc = ctx.enter_context(tc.tile_pool(name="ps_acc", bufs=1,
                                                space="PSUM"))

        ident = consts.tile([P, P], F32)
        make_identity(nc, ident)

        # K_aug / V_aug: [p, kc, 0:256] = row kc*P+p; col 256 = 1.0
        k_aug = big.tile([P, KC, DA], F32R)
        v_aug = big.tile([P, KC, DA], F32R)
        kt_sb = big.tile([P, DC, K], F32R)

        def load_aug(mat_dram, mat_aug):
            stage = raw.tile([P, KC, DA], F32, tag="raw")
            nc.sync.dma_start(stage[:, :, 0:D],
                              mat_dram.rearrange("(kc p) d -> p kc d", p=P))
            nc.vector.memset(stage[:, :, D:DA], 1.0)
            nc.vector.tensor_copy(mat_aug[:], stage[:])

        CH = 4  # k-chunks per load chunk
        for c0 in range(0, KC, CH):
            stage_c = raw.tile([P, CH, DA], F32, tag="rawc", bufs=2)
            nc.sync.dma_start(
                stage_c[:, :, 0:D],
                k_dram[c0 * P:(c0 + CH) * P, :].rearrange(
                    "(kc p) d -> p kc d", p=P))
            nc.vector.memset(stage_c[:, :, D:DA], 1.0)
            nc.vector.tensor_copy(k_aug[:, c0:c0 + CH, :], stage_c[:])
            for kc in range(c0, c0 + CH):
                for dc in range(DC):
                    tp = ps_st.tile([P, P], F32, tag="st")
                    nc.tensor.transpose(
                        tp, k_aug[:, kc, dc * P:(dc + 1) * P].bitcast(F32),
                        ident)
                    nc.scalar.copy(kt_sb[:, dc, kc * P:(kc + 1) * P], tp)

        first_rep = True
        for _ in range(reps):
            # Q^T: qt[p, dc, t] = q[t, dc*P+p]; updated in place each step.
            q_raw = raw.tile([P, n_tc, D], F32, tag="raw")
            nc.sync.dma_start(q_raw[:],
                              q_dram.rearrange("(tc p) d -> p tc d", p=P))
            qts = [qt_pool.tile([P, DC, tt_sz], F32R, tag=f"qt{tt}",
                                name=f"qt{tt}")
                   for tt in range(t_tiles)]
            for tcn in range(n_tc):
                tt, off = divmod(tcn * P, tt_sz)
                for dc in range(DC):
                    tp = ps_st.tile([P, P], F32, tag="st")
                    nc.tensor.transpose(tp, q_raw[:, tcn, dc * P:(dc + 1) * P],
                                        ident)
                    nc.scalar.copy(qts[tt][:, dc, off:off + P], tp)
            if first_rep:
                first_rep = False
                load_aug(v_dram, v_aug)  # needed only in the last step

            pending = None  # lazily emitted epilogue of previous t-tile
            for step in range(NSTEPS):
                last = step == NSTEPS - 1
                mat = v_aug if last else k_aug

                for tt in range(t_tiles):
                    if pending is not None and pending[1] == tt:
                        # epilogue writes qt[tt]; must precede this tile's
                        # scores in program order
                        pending[0]()
                        pending = None
                    qt = qts[tt]
                    acc = ps_acc.tile([P, n_sub, DA], F32, tag="acc",
                                      padded_shape=[P, n_sub, 512])

                    def qupd(pet, pkc, acc=acc, mat=mat):
                        for j in range(n_sub):
                            nc.tensor.matmul(
                                acc[:, j, :],
                                pet[:, bass.ts(j, P)],
                                mat[:, pkc, :],
                                start=(pkc == 0), stop=(pkc == KC - 1))

                    prev = None
                    for kc in range(KC):
                        st = ps_st.tile([P, tt_sz], F32, tag="st")
                        for dc in range(DC):
                            nc.tensor.matmul(
                                st,
                                kt_sb[:, dc, kc * P:(kc + 1) * P],
                                qt[:, dc, :],
                                start=(dc == 0), stop=(dc == DC - 1))
                        et = et_pool.tile([P, tt_sz], F32R, tag="et", bufs=6)
                        nc.scalar.activation(et, st, EXP, scale=beta)
                        if prev is not None:
                            qupd(*prev)
                        prev = (et, kc)
                        if kc == 1 and pending is not None:
                            # previous tile's epilogue hides under this loop
                            pending[0]()
                            pending = None
                    qupd(*prev)

                    def epilogue(acc=acc, tt=tt, qt=qts[tt], last=last):
                        # single fast ACT copy releases the PSUM accumulator
                        acc_sb = small.tile([P, n_sub, DA], F32, tag="accs",
                                            bufs=2)
                        nc.vector.tensor_copy(acc_sb, acc)
                        for j in range(n_sub):
                            row0 = tt * tt_sz + j * P
                            rcp = small.tile([P, 1], F32, tag="rcp")
                            nc.vector.reciprocal(rcp, acc_sb[:, j, D:D + 1])
                            if not last:
                                qn = small.tile([P, D], F32, tag="qn")
                                nc.vector.tensor_scalar_mul(
                                    qn, acc_sb[:, j, 0:D], rcp)
                                tp = ps_st.tile([P, D], F32, tag="st")
                                for dc in range(DC):
                                    nc.tensor.transpose(
                                        tp[:, dc * P:(dc + 1) * P],
                                        qn[:, dc * P:(dc + 1) * P], ident)
                                for dc in range(DC):
                                    nc.vector.tensor_copy(
                                        qt[:, dc, j * P:(j + 1) * P],
                                        tp[:, dc * P:(dc + 1) * P])
                            else:
                                ret_sb = ostage.tile([P, D], F32, tag="ret")
                                nc.vector.tensor_scalar_mul(
                                    ret_sb, acc_sb[:, j, 0:D], rcp)
                                nc.sync.dma_start(
                                    retr_dram[row0:row0 + P, :], ret_sb)
                                # attention rows, natural orientation
                                for kb in range(K // 512):
                                    s3n = ps_st.tile([P, 512], F32, tag="st")
                                    for dc in range(DC):
                                        nc.tensor.matmul(
                                            s3n,
                                            qt[:, dc, j * P:(j + 1) * P],
                                            kt_sb[:, dc,
                                                  kb * 512:(kb + 1) * 512],
                                            start=(dc == 0),
                                            stop=(dc == DC - 1))
                                    a_sb = ostage.tile([P, 512], F32,
                                                       tag="attn", bufs=6)
                                    nc.scalar.activation(a_sb, s3n, EXP,
                                                         scale=beta)
                                    nc.vector.tensor_scalar_mul(a_sb, a_sb,
                                                                rcp)
                                    nc.sync.dma_start(
                                        attn_dram[row0:row0 + P,
                                                  kb * 512:(kb + 1) * 512],
                                        a_sb)

                    pending = (epilogue, tt)
            if pending is not None:
                pending[0]()

    nc.compile()
    return nc


def kernel(query, keys, values, log_beta):
    beta = float(np.exp(np.float32(log_beta)))
    nc = build_kernel(beta)
    qf = np.ascontiguousarray(
        np.asarray(query, dtype=np.float32).reshape(B * TFULL, D))
    keys = np.ascontiguousarray(np.asarray(keys, dtype=np.float32))
    values = np.ascontiguousarray(np.asarray(values, dtype=np.float32))
    in_maps = [
        {"query": qf[c * T:(c + 1) * T], "keys": keys, "values": values}
        for c in range(N_CORES)
    ]
    res = run_bass_kernel_spmd(nc, in_maps, core_ids=list(range(N_CORES)))
    retrieved = np.concatenate(
        [res.results[c]["retrieved"] for c in range(N_CORES)], axis=0)
    attn = np.concatenate(
        [res.results[c]["attn"] for c in range(N_CORES)], axis=0)
    return (retrieved.reshape(B, TFULL, D).astype(np.float32),
            attn.reshape(B, TFULL, K).astype(np.float32))


# revision 25
# speedup vs baseline: 1.0059x; 1.0031x over previous
"""Trainium2 Bass kernel for iterative Hopfield associative memory retrieval.

Reference computation (per problem):
    for 3 steps: scores = q @ K^T * beta; attn = softmax(scores); q = attn @ K
    retrieved = attn @ V
    returns (retrieved, attn)

Sharding: data-parallel over the flattened (batch*token) dim across 8 cores;
keys/values replicated. Each core handles 2048 tokens.

Per-core kernel strategy (v2):
  - Scores are computed transposed (S^T [k, t]) so exp(S^T) tiles have the
    contraction dim k on partitions and feed the q-update / retrieval matmuls
    as stationary operands directly.
  - K and V are augmented with a ones column (K_aug [k, 257]) used as the
    moving operand: each accumulation produces the updated q (natural layout)
    PLUS the softmax row-sum in column 256 — no separate row-sum matmuls.
  - q-update: normalize by 1/rowsum (per-partition scalar), transpose back to
    [d, t] for the next step's scores.
  - Final step: the augmented-V accumulation yields retrieved (natural) +
    rowsums; attention is recomputed in natural [t, k] orientation with
    exp(beta*s - ln(rowsum)) fused in one ACT pass and DMA'd out directly.
  - All matmuls run float32r (full-rate fp32 streaming), fp32 PSUM accum.
  - Emission is software-pipelined: scores(kc+1) is emitted before the
    q-update of kc; per-tile epilogues are emitted after the next tile's
    main loop so the PE never waits on the normalize chain.
"""

from contextlib import ExitStack

import numpy as np

import concourse.bass as bass
import concourse.tile as tile
from concourse import bacc, mybir
from concourse.bass_utils import run_bass_kernel_spmd
from concourse.masks import make_identity

N_CORES = 8
B, TFULL, D = 2, 8192, 256
K = 4096
T = B * TFULL // N_CORES  # tokens per core = 2048
P = 128
DA = D + 2  # ones column (-> rowsum) + pad to even (fp32r ISA rule)
TT = 512  # token tile
KC = K // P  # 32 key chunks
DC = D // P  # 2 dim chunks
NSTEPS = 3

F32 = mybir.dt.float32
F32R = mybir.dt.float32r
EXP = mybir.ActivationFunctionType.Exp
LOG = mybir.ActivationFunctionType.Ln


def build_kernel(beta: float, n_tok: int = T, reps: int = 1):
    t_tiles = max(1, n_tok // TT)
    tt_sz = min(TT, n_tok)
    n_sub = tt_sz // P
    n_tc = n_tok // P
    nc = bacc.Bacc("TRN2", target_bir_lowering=False, debug=False,
                   num_devices=N_CORES)
    q_dram = nc.dram_tensor("query", [n_tok, D], F32, kind="ExternalInput").ap()
    k_dram = nc.dram_tensor("keys", [K, D], F32, kind="ExternalInput").ap()
    v_dram = nc.dram_tensor("values", [K, D], F32, kind="ExternalInput").ap()
    retr_dram = nc.dram_tensor("retrieved", [n_tok, D], F32,
                               kind="ExternalOutput").ap()
    attn_dram = nc.dram_tensor("attn", [n_tok, K], F32,
                               kind="ExternalOutput").ap()

    with tile.TileContext(nc) as tc, ExitStack() as ctx, \
            nc.allow_low_precision(reason="fp32r rounding is intended"):
        consts = ctx.enter_context(tc.tile_pool(name="consts", bufs=1))
        big = ctx.enter_context(tc.tile_pool(name="big", bufs=1))
        raw = ctx.enter_context(tc.tile_pool(name="raw", bufs=1))
        qt_pool = ctx.enter_context(tc.tile_pool(name="qtp", bufs=1))
        et_pool = ctx.enter_context(tc.tile_pool(name="etp", bufs=4))
        small = ctx.enter_context(tc.tile_pool(name="small", bufs=4))
        ostage = ctx.enter_context(tc.tile_pool(name="ostage", bufs=4))
        ps_st = ctx.enter_context(tc.tile_pool(name="ps_st", bufs=3,
                                               space="PSUM"))
        ps_acc = ctx.enter_context(tc.tile_pool(name="ps_acc", bufs=1,
                                                space="PSUM"))

        ident = consts.tile([P, P], F32)
        make_identity(nc, ident)

        # K_aug / V_aug: [p, kc, 0:256] = row kc*P+p; col 256 = 1.0
        k_aug = big.tile([P, KC, DA], F32R)
        v_aug = big.tile([P, KC, DA], F32R)
        kt_sb = big.tile([P, DC, K], F32R)

        def load_aug(mat_dram, mat_aug):
            stage = raw.tile([P, KC, DA], F32, tag="raw")
            nc.sync.dma_start(stage[:, :, 0:D],
                              mat_dram.rearrange("(kc p) d -> p kc d", p=P))
            nc.vector.memset(stage[:, :, D:DA], 1.0)
            nc.vector.tensor_copy(mat_aug[:], stage[:])

        CH = 4  # k-chunks per load chunk
        for c0 in range(0, KC, CH):
            stage_c = raw.tile([P, CH, DA], F32, tag="rawc", bufs=2)
            nc.sync.dma_start(
                stage_c[:, :, 0:D],
                k_dram[c0 * P:(c0 + CH) * P, :].rearrange(
                    "(kc p) d -> p kc d", p=P))
            nc.vector.memset(stage_c[:, :, D:DA], 1.0)
            nc.vector.tensor_copy(k_aug[:, c0:c0 + CH, :], stage_c[:])
            for kc in range(c0, c0 + CH):
                for dc in range(DC):
                    tp = ps_st.tile([P, P], F32, tag="st")
                    nc.tensor.transpose(
                        tp, k_aug[:, kc, dc * P:(dc + 1) * P].bitcast(F32),
                        ident)
                    nc.scalar.copy(kt_sb[:, dc, kc * P:(kc + 1) * P], tp)

        first_rep = True
        for _ in range(reps):
            # Q^T: qt[p, dc, t] = q[t, dc*P+p]; updated in place each step.
            q_raw = raw.tile([P, n_tc, D], F32, tag="raw")
            nc.sync.dma_start(q_raw[:],
                              q_dram.rearrange("(tc p) d -> p tc d", p=P))
            qts = [qt_pool.tile([P, DC, tt_sz], F32R, tag=f"qt{tt}",
                                name=f"qt{tt}")
                   for tt in range(t_tiles)]
            for tcn in range(n_tc):
                tt, off = divmod(tcn * P, tt_sz)
                for dc in range(DC):
                    tp = ps_st.tile([P, P], F32, tag="st")
                    nc.tensor.transpose(tp, q_raw[:, tcn, dc * P:(dc + 1) * P],
                                        ident)
                    nc.scalar.copy(qts[tt][:, dc, off:off + P], tp)
            if first_rep:
                first_rep = False
                load_aug(v_dram, v_aug)  # needed only in the last step

            pending = None  # lazily emitted epilogue of previous t-tile
            for step in range(NSTEPS):
                last = step == NSTEPS - 1
                mat = v_aug if last else k_aug

                for tt in range(t_tiles):
                    if pending is not None and pending[1] == tt:
                        # epilogue writes qt[tt]; must precede this tile's
                        # scores in program order
                        pending[0]()
                        pending = None
                    qt = qts[tt]
                    acc = ps_acc.tile([P, n_sub, DA], F32, tag="acc",
                                      padded_shape=[P, n_sub, 512])

                    def qupd(pet, pkc, acc=acc, mat=mat):
                        for j in range(n_sub):
                            nc.tensor.matmul(
                                acc[:, j, :],
                                pet[:, bass.ts(j, P)],
                                mat[:, pkc, :],
                                start=(pkc == 0), stop=(pkc == KC - 1))

                    prev = None
                    for kc in range(KC):
                        st = ps_st.tile([P, tt_sz], F32, tag="st")
                        for dc in range(DC):
                            nc.tensor.matmul(
                                st,
                                kt_sb[:, dc, kc * P:(kc + 1) * P],
                                qt[:, dc, :],
                                start=(dc == 0), stop=(dc == DC - 1))
                        et = et_pool.tile([P, tt_sz], F32R, tag="et", bufs=6)
                        nc.scalar.activation(et, st, EXP, scale=beta)
                        if prev is not None:
                            qupd(*prev)
                        prev = (et, kc)
                        if kc == 1 and pending is not None:
                            # previous tile's epilogue hides under this loop
                            pending[0]()
                            pending = None
                    qupd(*prev)

                    def epilogue(acc=acc, tt=tt, qt=qts[tt], last=last):
                        # single fast ACT copy releases the PSUM accumulator
                        acc_sb = small.tile([P, n_sub, DA], F32, tag="accs",
                                            bufs=2)
                        nc.vector.tensor_copy(acc_sb, acc)
                        for j in range(n_sub):
                            row0 = tt * tt_sz + j * P
                            rcp = small.tile([P, 1], F32, tag="rcp")
                            nc.vector.reciprocal(rcp, acc_sb[:, j, D:D + 1])
                            if not last:
                                qn = small.tile([P, D], F32, tag="qn")
                                nc.vector.tensor_scalar_mul(
                                    qn, acc_sb[:, j, 0:D], rcp)
                                tp = ps_st.tile([P, D], F32, tag="st")
                                for dc in range(DC):
                                    nc.tensor.transpose(
                                        tp[:, dc * P:(dc + 1) * P],
                                        qn[:, dc * P:(dc + 1) * P], ident)
                                for dc in range(DC):
                                    nc.vector.tensor_copy(
                                        qt[:, dc, j * P:(j + 1) * P],
                                        tp[:, dc * P:(dc + 1) * P])
                            else:
                                ret_sb = ostage.tile([P, D], F32, tag="ret")
                                nc.vector.tensor_scalar_mul(
                                    ret_sb, acc_sb[:, j, 0:D], rcp)
                                nc.sync.dma_start(
                                    retr_dram[row0:row0 + P, :], ret_sb)
                                # attention rows, natural orientation
                                for kb in range(K // 512):
                                    s3n = ps_st.tile([P, 512], F32, tag="st")
                                    for dc in range(DC):
                                        nc.tensor.matmul(
                                            s3n,
                                            qt[:, dc, j * P:(j + 1) * P],
                                            kt_sb[:, dc,
                                                  kb * 512:(kb + 1) * 512],
                                            start=(dc == 0),
                                            stop=(dc == DC - 1))
                                    a_sb = ostage.tile([P, 512], F32,
                                                       tag="attn", bufs=6)
                                    nc.scalar.activation(a_sb, s3n, EXP,
                                                         scale=beta)
                                    nc.vector.tensor_scalar_mul(a_sb, a_sb,
                                                                rcp)
                                    nc.sync.dma_start(
                                        attn_dram[row0:row0 + P,
                                                  kb * 512:(kb + 1) * 512],
                                        a_sb)

                    pending = (epilogue, tt)
            if pending is not None:
                pending[0]()

    nc.compile()
    return nc


def kernel(query, keys, values, log_beta):
    beta = float(np.exp(np.float32(log_beta)))
    nc = build_kernel(beta)
    qf = np.ascontiguousarray(
        np.asarray(query, dtype=np.float32).reshape(B * TFULL, D))
    keys = np.ascontiguousarray(np.asarray(keys, dtype=np.float32))
    values = np.ascontiguousarray(np.asarray(values, dtype=np.float32))
    in_maps = [
        {"query": qf[c * T:(c + 1) * T], "keys": keys, "values": values}
        for c in range(N_CORES)
    ]
    res = run_bass_kernel_spmd(nc, in_maps, core_ids=list(range(N_CORES)))
    retrieved = np.concatenate(
        [res.results[c]["retrieved"] for c in range(N_CORES)], axis=0)
    attn = np.concatenate(
        [res.results[c]["attn"] for c in range(N_CORES)], axis=0)
    return (retrieved.reshape(B, TFULL, D).astype(np.float32),
            attn.reshape(B, TFULL, K).astype(np.float32))


# revision 30
# speedup vs baseline: 1.0276x; 1.0216x over previous
"""Trainium2 Bass kernel for iterative Hopfield associative memory retrieval.

Reference computation (per problem):
    for 3 steps: scores = q @ K^T * beta; attn = softmax(scores); q = attn @ K
    retrieved = attn @ V
    returns (retrieved, attn)

Sharding: data-parallel over the flattened (batch*token) dim across 8 cores;
keys/values replicated. Each core handles 2048 tokens.

Per-core kernel strategy:
  - Scores are computed transposed (S^T [k, t]) so exp(S^T) tiles have the
    contraction dim k on partitions and feed the q-update / retrieval matmuls
    as stationary operands directly.
  - K and V are augmented with a ones column (K_aug [k, 257]) used as the
    moving operand: each accumulation produces the updated q (natural layout)
    PLUS the softmax row-sum in column 256 — no separate row-sum matmuls.
  - q-update: normalize by 1/rowsum (per-partition scalar), transpose back to
    [d, t] for the next step's scores.
  - Final step: the augmented-V accumulation yields retrieved (natural) +
    rowsums; attention is recomputed in natural [t, k] orientation, exp'd on
    the scalar engine (no table-switching functions), normalized by a
    per-partition 1/rowsum multiply on the vector engine, and DMA'd out.
  - All matmuls run float32r (full-rate fp32 streaming), fp32 PSUM accum.
  - Emission is software-pipelined: scores(kc+1) is emitted before the
    q-update of kc; per-tile epilogues are emitted after the next tile's
    main loop so the PE never waits on the normalize chain.
"""

from contextlib import ExitStack

import numpy as np

import concourse.bass as bass
import concourse.tile as tile
from concourse import bacc, mybir
from concourse.bass_utils import run_bass_kernel_spmd
from concourse.masks import make_identity

N_CORES = 8
B, TFULL, D = 2, 8192, 256
K = 4096
T = B * TFULL // N_CORES  # tokens per core = 2048
P = 128
DA = D + 2  # ones column (-> rowsum) + pad to even (fp32r ISA rule)
TT = 512  # token tile
KC = K // P  # 32 key chunks
DC = D // P  # 2 dim chunks
NSTEPS = 3

F32 = mybir.dt.float32
F32R = mybir.dt.float32r
EXP = mybir.ActivationFunctionType.Exp


def build_kernel(beta: float, n_tok: int = T, reps: int = 1):
    t_tiles = max(1, n_tok // TT)
    tt_sz = min(TT, n_tok)
    n_sub = tt_sz // P
    n_tc = n_tok // P
    nc = bacc.Bacc("TRN2", target_bir_lowering=False, debug=False,
                   num_devices=N_CORES)
    q_dram = nc.dram_tensor("query", [n_tok, D], F32, kind="ExternalInput").ap()
    k_dram = nc.dram_tensor("keys", [K, D], F32, kind="ExternalInput").ap()
    v_dram = nc.dram_tensor("values", [K, D], F32, kind="ExternalInput").ap()
    retr_dram = nc.dram_tensor("retrieved", [n_tok, D], F32,
                               kind="ExternalOutput").ap()
    attn_dram = nc.dram_tensor("attn", [n_tok, K], F32,
                               kind="ExternalOutput").ap()

    with tile.TileContext(nc) as tc, ExitStack() as ctx, \
            nc.allow_low_precision(reason="fp32r rounding is intended"):
        consts = ctx.enter_context(tc.tile_pool(name="consts", bufs=1))
        big = ctx.enter_context(tc.tile_pool(name="big", bufs=1))
        raw = ctx.enter_context(tc.tile_pool(name="raw", bufs=1))
        qt_pool = ctx.enter_context(tc.tile_pool(name="qtp", bufs=1))
        et_pool = ctx.enter_context(tc.tile_pool(name="etp", bufs=4))
        small = ctx.enter_context(tc.tile_pool(name="small", bufs=4))
        ostage = ctx.enter_context(tc.tile_pool(name="ostage", bufs=4))
        ps_st = ctx.enter_context(tc.tile_pool(name="ps_st", bufs=4,
                                               space="PSUM"))
        ps_acc = ctx.enter_context(tc.tile_pool(name="ps_acc", bufs=1,
                                                space="PSUM"))

        ident = consts.tile([P, P], F32)
        make_identity(nc, ident)

        # K_aug / V_aug: [p, kc, 0:256] = row kc*P+p; col 256 = 1.0
        k_aug = big.tile([P, KC, DA], F32R)
        v_aug = big.tile([P, KC, DA], F32R)
        kt_sb = big.tile([P, DC, K], F32R)

        def load_aug(mat_dram, mat_aug):
            stage = raw.tile([P, KC, DA], F32, tag="raw")
            nc.sync.dma_start(stage[:, :, 0:D],
                              mat_dram.rearrange("(kc p) d -> p kc d", p=P))
            nc.vector.memset(stage[:, :, D:DA], 1.0)
            nc.vector.tensor_copy(mat_aug[:], stage[:])

        CH = 2  # k-chunks per load chunk
        for c0 in range(0, KC, CH):
            stage_c = raw.tile([P, CH, DA], F32, tag="rawc", bufs=4)
            nc.sync.dma_start(
                stage_c[:, :, 0:D],
                k_dram[c0 * P:(c0 + CH) * P, :].rearrange(
                    "(kc p) d -> p kc d", p=P))
            nc.vector.memset(stage_c[:, :, D:DA], 1.0)
            nc.vector.tensor_copy(k_aug[:, c0:c0 + CH, :], stage_c[:])
            for kc in range(c0, c0 + CH):
                for dc in range(DC):
                    tp = ps_st.tile([P, P], F32, tag="st")
                    nc.tensor.transpose(
                        tp, k_aug[:, kc, dc * P:(dc + 1) * P].bitcast(F32),
                        ident)
                    nc.scalar.copy(kt_sb[:, dc, kc * P:(kc + 1) * P], tp)

        first_rep = True
        for _ in range(reps):
            # Q^T: qt[p, dc, t] = q[t, dc*P+p]; updated in place each step.
            q_raw = raw.tile([P, n_tc, D], F32, tag="raw")
            nc.sync.dma_start(q_raw[:],
                              q_dram.rearrange("(tc p) d -> p tc d", p=P))
            qts = [qt_pool.tile([P, DC, tt_sz], F32R, tag=f"qt{tt}",
                                name=f"qt{tt}")
                   for tt in range(t_tiles)]
            for tcn in range(n_tc):
                tt, off = divmod(tcn * P, tt_sz)
                for dc in range(DC):
                    tp = ps_st.tile([P, P], F32, tag="st")
                    nc.tensor.transpose(tp, q_raw[:, tcn, dc * P:(dc + 1) * P],
                                        ident)
                    nc.scalar.copy(qts[tt][:, dc, off:off + P], tp)
            if first_rep:
                first_rep = False
                load_aug(v_dram, v_aug)  # needed only in the last step

            pending = None  # lazily emitted epilogue of previous t-tile
            for step in range(NSTEPS):
                last = step == NSTEPS - 1
                mat = v_aug if last else k_aug

                segs = [(tt, 0, tt_sz) for tt in range(t_tiles)]
                if last and tt_sz >= 2 * P:
                    # split the final tile so the serial tail epilogue is
                    # half the size
                    tti, off, sz = segs[-1]
                    segs[-1:] = [(tti, off, sz // 2),
                                 (tti, off + sz // 2, sz - sz // 2)]
                for tti, off, sz in segs:
                    if pending is not None and pending[1] == tti:
                        # epilogue writes qt[tti]; must precede this tile's
                        # scores in program order
                        pending[0]()
                        pending = None
                    qt = qts[tti]
                    n_sb = sz // P
                    acc = ps_acc.tile([P, n_sb, DA], F32, tag="acc",
                                      padded_shape=[P, n_sub, 512])

                    def qupd(pet, pkc, acc=acc, mat=mat, n_sb=n_sb):
                        for j in range(n_sb):
                            nc.tensor.matmul(
                                acc[:, j, :],
                                pet[:, bass.ts(j, P)],
                                mat[:, pkc, :],
                                start=(pkc == 0), stop=(pkc == KC - 1))

                    prev = None
                    for kc in range(KC):
                        st = ps_st.tile([P, sz], F32, tag="st")
                        for dc in range(DC):
                            nc.tensor.matmul(
                                st,
                                kt_sb[:, dc, kc * P:(kc + 1) * P],
                                qt[:, dc, off:off + sz],
                                start=(dc == 0), stop=(dc == DC - 1))
                        et = et_pool.tile([P, sz], F32R, tag="et", bufs=6)
                        nc.scalar.activation(et, st, EXP, scale=beta)
                        if prev is not None:
                            qupd(*prev)
                        prev = (et, kc)
                        if kc == 1 and pending is not None:
                            # previous tile's epilogue hides under this loop
                            pending[0]()
                            pending = None
                    qupd(*prev)

                    def epilogue(acc=acc, tti=tti, off=off, n_sb=n_sb,
                                 qt=qt, last=last):
                        # single fast DVE copy releases the PSUM accumulator
                        acc_sb = small.tile([P, n_sb, DA], F32, tag="accs",
                                            bufs=3)
                        nc.vector.tensor_copy(acc_sb, acc)
                        for j in range(n_sb):
                            row0 = tti * tt_sz + off + j * P
                            qoff = off + j * P
                            rcp = small.tile([P, 1], F32, tag="rcp")
                            nc.vector.reciprocal(rcp, acc_sb[:, j, D:D + 1])
                            if not last:
                                qn = small.tile([P, D], F32, tag="qn")
                                nc.vector.tensor_scalar_mul(
                                    qn, acc_sb[:, j, 0:D], rcp)
                                tp = ps_st.tile([P, D], F32, tag="st")
                                for dc in range(DC):
                                    nc.tensor.transpose(
                                        tp[:, dc * P:(dc + 1) * P],
                                        qn[:, dc * P:(dc + 1) * P], ident)
                                for dc in range(DC):
                                    nc.vector.tensor_copy(
                                        qt[:, dc, qoff:qoff + P],
                                        tp[:, dc * P:(dc + 1) * P])
                            else:
                                ret_sb = ostage.tile([P, D], F32, tag="ret")
                                nc.vector.tensor_scalar_mul(
                                    ret_sb, acc_sb[:, j, 0:D], rcp)
                                nc.sync.dma_start(
                                    retr_dram[row0:row0 + P, :], ret_sb)
                                # attention rows, natural orientation
                                for kb in range(K // 512):
                                    s3n = ps_st.tile([P, 512], F32, tag="st")
                                    for dc in range(DC):
                                        nc.tensor.matmul(
                                            s3n,
                                            qt[:, dc, qoff:qoff + P],
                                            kt_sb[:, dc,
                                                  kb * 512:(kb + 1) * 512],
                                            start=(dc == 0),
                                            stop=(dc == DC - 1))
                                    a_sb = ostage.tile([P, 512], F32,
                                                       tag="attn", bufs=6)
                                    nc.scalar.activation(a_sb, s3n, EXP,
                                                         scale=beta)
                                    nc.vector.tensor_scalar_mul(a_sb, a_sb,
                                                                rcp)
                                    nc.sync.dma_start(
                                        attn_dram[row0:row0 + P,
                                                  kb * 512:(kb + 1) * 512],
                                        a_sb)

                    pending = (epilogue, tti)
            if pending is not None:
                pending[0]()

    nc.compile()
    return nc


def kernel(query, keys, values, log_beta):
    beta = float(np.exp(np.float32(log_beta)))
    nc = build_kernel(beta)
    qf = np.ascontiguousarray(
        np.asarray(query, dtype=np.float32).reshape(B * TFULL, D))
    keys = np.ascontiguousarray(np.asarray(keys, dtype=np.float32))
    values = np.ascontiguousarray(np.asarray(values, dtype=np.float32))
    in_maps = [
        {"query": qf[c * T:(c + 1) * T], "keys": keys, "values": values}
        for c in range(N_CORES)
    ]
    res = run_bass_kernel_spmd(nc, in_maps, core_ids=list(range(N_CORES)))
    retrieved = np.concatenate(
        [res.results[c]["retrieved"] for c in range(N_CORES)], axis=0)
    attn = np.concatenate(
        [res.results[c]["attn"] for c in range(N_CORES)], axis=0)
    return (retrieved.reshape(B, TFULL, D).astype(np.float32),
            attn.reshape(B, TFULL, K).astype(np.float32))


# revision 32
# speedup vs baseline: 1.3003x; 1.2654x over previous
"""Trainium2 Bass kernel for iterative Hopfield associative memory retrieval.

Reference computation (per problem):
    for 3 steps: scores = q @ K^T * beta; attn = softmax(scores); q = attn @ K
    retrieved = attn @ V
    returns (retrieved, attn)

Sharding: data-parallel over the flattened (batch*token) dim across 8 cores;
keys/values replicated. Each core handles 2048 tokens.

Per-core kernel strategy:
  - Scores are computed transposed (S^T [k, t]) so exp(S^T) tiles have the
    contraction dim k on partitions and feed the q-update / retrieval matmuls
    as stationary operands directly.
  - K and V are augmented with a ones column (K_aug [k, 257]) used as the
    moving operand: each accumulation produces the updated q (natural layout)
    PLUS the softmax row-sum in column 256 — no separate row-sum matmuls.
  - q-update: normalize by 1/rowsum (per-partition scalar), transpose back to
    [d, t] for the next step's scores.
  - Final step: the augmented-V accumulation yields retrieved (natural) +
    rowsums; attention is recomputed in natural [t, k] orientation, exp'd on
    the scalar engine (no table-switching functions), normalized by a
    per-partition 1/rowsum multiply on the vector engine, and DMA'd out.
  - All matmuls run float32r (full-rate fp32 streaming), fp32 PSUM accum.
  - Emission is software-pipelined: scores(kc+1) is emitted before the
    q-update of kc; per-tile epilogues are emitted after the next tile's
    main loop so the PE never waits on the normalize chain.
"""

from contextlib import ExitStack

import numpy as np

import concourse.bass as bass
import concourse.tile as tile
from concourse import bacc, mybir
from concourse.bass_utils import run_bass_kernel_spmd
from concourse.masks import make_identity

N_CORES = 8
B, TFULL, D = 2, 8192, 256
K = 4096
T = B * TFULL // N_CORES  # tokens per core = 2048
P = 128
DA = D + 2  # ones column (-> rowsum) + pad to even (fp32r ISA rule)
TT = 512  # token tile
KC = K // P  # 32 key chunks
DC = D // P  # 2 dim chunks
NSTEPS = 3

F32 = mybir.dt.float32
BF16 = mybir.dt.bfloat16
EXP = mybir.ActivationFunctionType.Exp


def build_kernel(beta: float, n_tok: int = T, reps: int = 1):
    t_tiles = max(1, n_tok // TT)
    tt_sz = min(TT, n_tok)
    n_sub = tt_sz // P
    n_tc = n_tok // P
    nc = bacc.Bacc("TRN2", target_bir_lowering=False, debug=False,
                   num_devices=N_CORES)
    q_dram = nc.dram_tensor("query", [n_tok, D], F32, kind="ExternalInput").ap()
    k_dram = nc.dram_tensor("keys", [K, D], F32, kind="ExternalInput").ap()
    v_dram = nc.dram_tensor("values", [K, D], F32, kind="ExternalInput").ap()
    retr_dram = nc.dram_tensor("retrieved", [n_tok, D], F32,
                               kind="ExternalOutput").ap()
    attn_dram = nc.dram_tensor("attn", [n_tok, K], F32,
                               kind="ExternalOutput").ap()

    with tile.TileContext(nc) as tc, ExitStack() as ctx, \
            nc.allow_low_precision(reason="fp32r rounding is intended"):
        consts = ctx.enter_context(tc.tile_pool(name="consts", bufs=1))
        big = ctx.enter_context(tc.tile_pool(name="big", bufs=1))
        raw = ctx.enter_context(tc.tile_pool(name="raw", bufs=1))
        qt_pool = ctx.enter_context(tc.tile_pool(name="qtp", bufs=1))
        et_pool = ctx.enter_context(tc.tile_pool(name="etp", bufs=4))
        small = ctx.enter_context(tc.tile_pool(name="small", bufs=4))
        ostage = ctx.enter_context(tc.tile_pool(name="ostage", bufs=4))
        ps_st = ctx.enter_context(tc.tile_pool(name="ps_st", bufs=4,
                                               space="PSUM"))
        ps_acc = ctx.enter_context(tc.tile_pool(name="ps_acc", bufs=1,
                                                space="PSUM"))

        ident = consts.tile([P, P], F32)
        make_identity(nc, ident)
        ident_bf = consts.tile([P, P], BF16)
        nc.vector.tensor_copy(ident_bf, ident)

        # K_aug / V_aug: [p, kc, 0:256] = row kc*P+p; col 256 = 1.0
        k_aug = big.tile([P, KC, DA], BF16)
        v_aug = big.tile([P, KC, DA], BF16)
        kt_sb = big.tile([P, DC, K], BF16)

        def load_aug(mat_dram, mat_aug):
            stage = raw.tile([P, KC, DA], F32, tag="raw")
            nc.sync.dma_start(stage[:, :, 0:D],
                              mat_dram.rearrange("(kc p) d -> p kc d", p=P))
            nc.vector.memset(stage[:, :, D:DA], 1.0)
            nc.vector.tensor_copy(mat_aug[:], stage[:])

        CH = 2  # k-chunks per load chunk
        for c0 in range(0, KC, CH):
            stage_c = raw.tile([P, CH, DA], F32, tag="rawc", bufs=6)
            nc.sync.dma_start(
                stage_c[:, :, 0:D],
                k_dram[c0 * P:(c0 + CH) * P, :].rearrange(
                    "(kc p) d -> p kc d", p=P))
            nc.vector.memset(stage_c[:, :, D:DA], 1.0)
            nc.vector.tensor_copy(k_aug[:, c0:c0 + CH, :], stage_c[:])
            for kc in range(c0, c0 + CH):
                for dc in range(DC):
                    tp = ps_st.tile([P, P], BF16, tag="st")
                    nc.tensor.transpose(
                        tp, k_aug[:, kc, dc * P:(dc + 1) * P], ident_bf)
                    nc.scalar.copy(kt_sb[:, dc, kc * P:(kc + 1) * P], tp)

        first_rep = True
        for _ in range(reps):
            # Q^T: qt[p, dc, t] = q[t, dc*P+p]; updated in place each step.
            q_raw = raw.tile([P, n_tc, D], F32, tag="raw")
            nc.sync.dma_start(q_raw[:],
                              q_dram.rearrange("(tc p) d -> p tc d", p=P))
            qts = [qt_pool.tile([P, DC, tt_sz], BF16, tag=f"qt{tt}",
                                name=f"qt{tt}")
                   for tt in range(t_tiles)]
            for tcn in range(n_tc):
                tt, off = divmod(tcn * P, tt_sz)
                for dc in range(DC):
                    tp = ps_st.tile([P, P], F32, tag="st")
                    nc.tensor.transpose(tp, q_raw[:, tcn, dc * P:(dc + 1) * P],
                                        ident)
                    nc.scalar.copy(qts[tt][:, dc, off:off + P], tp)
            if first_rep:
                first_rep = False
                load_aug(v_dram, v_aug)  # needed only in the last step

            pending = None  # lazily emitted epilogue of previous t-tile
            for step in range(NSTEPS):
                last = step == NSTEPS - 1
                mat = v_aug if last else k_aug

                segs = [(tt, 0, tt_sz) for tt in range(t_tiles)]
                if last and tt_sz >= 2 * P:
                    # split the final tile so the serial tail epilogue is
                    # half the size
                    tti, off, sz = segs[-1]
                    segs[-1:] = [(tti, off, sz // 2),
                                 (tti, off + sz // 2, sz - sz // 2)]
                for tti, off, sz in segs:
                    if pending is not None and pending[1] == tti:
                        # epilogue writes qt[tti]; must precede this tile's
                        # scores in program order
                        pending[0]()
                        pending = None
                    qt = qts[tti]
                    n_sb = sz // P
                    acc = ps_acc.tile([P, n_sb, DA], F32, tag="acc",
                                      padded_shape=[P, n_sub, 512])

                    def qupd(pet, pkc, acc=acc, mat=mat, n_sb=n_sb):
                        for j in range(n_sb):
                            nc.tensor.matmul(
                                acc[:, j, :],
                                pet[:, bass.ts(j, P)],
                                mat[:, pkc, :],
                                start=(pkc == 0), stop=(pkc == KC - 1))

                    prev = None
                    for kc in range(KC):
                        st = ps_st.tile([P, sz], F32, tag="st")
                        for dc in range(DC):
                            nc.tensor.matmul(
                                st,
                                kt_sb[:, dc, kc * P:(kc + 1) * P],
                                qt[:, dc, off:off + sz],
                                start=(dc == 0), stop=(dc == DC - 1))
                        et = et_pool.tile([P, sz], BF16, tag="et", bufs=10)
                        nc.scalar.activation(et, st, EXP, scale=beta)
                        if prev is not None:
                            qupd(*prev)
                        prev = (et, kc)
                        if kc == 1 and pending is not None:
                            # previous tile's epilogue hides under this loop
                            pending[0]()
                            pending = None
                    qupd(*prev)

                    def epilogue(acc=acc, tti=tti, off=off, n_sb=n_sb,
                                 qt=qt, last=last):
                        # single fast DVE copy releases the PSUM accumulator
                        acc_sb = small.tile([P, n_sb, DA], F32, tag="accs",
                                            bufs=4)
                        nc.vector.tensor_copy(acc_sb, acc)
                        for j in range(n_sb):
                            row0 = tti * tt_sz + off + j * P
                            qoff = off + j * P
                            rcp = small.tile([P, 1], F32, tag="rcp")
                            nc.vector.reciprocal(rcp, acc_sb[:, j, D:D + 1])
                            if not last:
                                qn = small.tile([P, D], F32, tag="qn")
                                nc.vector.tensor_scalar_mul(
                                    qn, acc_sb[:, j, 0:D], rcp)
                                tp = ps_st.tile([P, D], F32, tag="st")
                                for dc in range(DC):
                                    nc.tensor.transpose(
                                        tp[:, dc * P:(dc + 1) * P],
                                        qn[:, dc * P:(dc + 1) * P], ident)
                                for dc in range(DC):
                                    nc.vector.tensor_copy(
                                        qt[:, dc, qoff:qoff + P],
                                        tp[:, dc * P:(dc + 1) * P])
                            else:
                                ret_sb = ostage.tile([P, D], F32, tag="ret")
                                nc.vector.tensor_scalar_mul(
                                    ret_sb, acc_sb[:, j, 0:D], rcp)
                                nc.sync.dma_start(
                                    retr_dram[row0:row0 + P, :], ret_sb)
                                # attention rows, natural orientation
                                for kb in range(K // 512):
                                    s3n = ps_st.tile([P, 512], F32, tag="st")
                                    for dc in range(DC):
                                        nc.tensor.matmul(
                                            s3n,
                                            qt[:, dc, qoff:qoff + P],
                                            kt_sb[:, dc,
                                                  kb * 512:(kb + 1) * 512],
                                            start=(dc == 0),
                                            stop=(dc == DC - 1))
                                    a_sb = ostage.tile([P, 512], F32,
                                                       tag="attn", bufs=8)
                                    nc.scalar.activation(a_sb, s3n, EXP,
                                                         scale=beta)
                                    nc.vector.tensor_scalar_mul(a_sb, a_sb,
                                                                rcp)
                                    nc.sync.dma_start(
                                        attn_dram[row0:row0 + P,
                                                  kb * 512:(kb + 1) * 512],
                                        a_sb)

                    pending = (epilogue, tti)
            if pending is not None:
                pending[0]()

    nc.compile()
    return nc


def kernel(query, keys, values, log_beta):
    beta = float(np.exp(np.float32(log_beta)))
    nc = build_kernel(beta)
    qf = np.ascontiguousarray(
        np.asarray(query, dtype=np.float32).reshape(B * TFULL, D))
    keys = np.ascontiguousarray(np.asarray(keys, dtype=np.float32))
    values = np.ascontiguousarray(np.asarray(values, dtype=np.float32))
    in_maps = [
        {"query": qf[c * T:(c + 1) * T], "keys": keys, "values": values}
        for c in range(N_CORES)
    ]
    res = run_bass_kernel_spmd(nc, in_maps, core_ids=list(range(N_CORES)))
    retrieved = np.concatenate(
        [res.results[c]["retrieved"] for c in range(N_CORES)], axis=0)
    attn = np.concatenate(
        [res.results[c]["attn"] for c in range(N_CORES)], axis=0)
    return (retrieved.reshape(B, TFULL, D).astype(np.float32),
            attn.reshape(B, TFULL, K).astype(np.float32))


# revision 36
# speedup vs baseline: 1.3159x; 1.0120x over previous
"""Trainium2 Bass kernel for iterative Hopfield associative memory retrieval.

Reference computation (per problem):
    for 3 steps: scores = q @ K^T * beta; attn = softmax(scores); q = attn @ K
    retrieved = attn @ V
    returns (retrieved, attn)

Sharding: data-parallel over the flattened (batch*token) dim across 8 cores;
keys/values replicated. Each core handles 2048 tokens.

Per-core kernel strategy:
  - Scores are computed transposed (S^T [k, t]) so exp(S^T) tiles have the
    contraction dim k on partitions and feed the q-update / retrieval matmuls
    as stationary operands directly.
  - K and V are augmented with a ones column (K_aug [k, 257]) used as the
    moving operand: each accumulation produces the updated q (natural layout)
    PLUS the softmax row-sum in column 256 — no separate row-sum matmuls.
  - q-update: normalize by 1/rowsum (per-partition scalar), transpose back to
    [d, t] for the next step's scores.
  - Final step: the augmented-V accumulation yields retrieved (natural) +
    rowsums; attention is recomputed in natural [t, k] orientation, exp'd on
    the scalar engine (no table-switching functions), normalized by a
    per-partition 1/rowsum multiply on the vector engine, and DMA'd out.
  - Matmul operands are bf16 (same 1 cycle/row as fp32r but with fast
    weight loads), accumulation in fp32 PSUM; all outputs stay fp32.
    Measured end-to-end error ~2e-3 rel-L2 (gate 2e-2): the peaked softmax
    cancels correlated rounding errors.
  - Emission is software-pipelined: scores(kc+1) is emitted before the
    q-update of kc; per-tile epilogues are emitted after the next tile's
    main loop so the PE never waits on the normalize chain.
"""

from contextlib import ExitStack

import numpy as np

import concourse.bass as bass
import concourse.tile as tile
from concourse import bacc, mybir
from concourse.bass_utils import run_bass_kernel_spmd
from concourse.masks import make_identity

N_CORES = 8
B, TFULL, D = 2, 8192, 256
K = 4096
T = B * TFULL // N_CORES  # tokens per core = 2048
P = 128
DA = D + 2  # ones column (-> rowsum) + pad to even element count
TT = 512  # token tile
KC = K // P  # 32 key chunks
DC = D // P  # 2 dim chunks
NSTEPS = 3

F32 = mybir.dt.float32
BF16 = mybir.dt.bfloat16
EXP = mybir.ActivationFunctionType.Exp


def build_kernel(beta: float, n_tok: int = T, reps: int = 1):
    t_tiles = max(1, n_tok // TT)
    tt_sz = min(TT, n_tok)
    n_sub = tt_sz // P
    n_tc = n_tok // P
    nc = bacc.Bacc("TRN2", target_bir_lowering=False, debug=False,
                   num_devices=N_CORES)
    q_dram = nc.dram_tensor("query", [n_tok, D], F32, kind="ExternalInput").ap()
    k_dram = nc.dram_tensor("keys", [K, D], F32, kind="ExternalInput").ap()
    v_dram = nc.dram_tensor("values", [K, D], F32, kind="ExternalInput").ap()
    retr_dram = nc.dram_tensor("retrieved", [n_tok, D], F32,
                               kind="ExternalOutput").ap()
    attn_dram = nc.dram_tensor("attn", [n_tok, K], F32,
                               kind="ExternalOutput").ap()

    with tile.TileContext(nc) as tc, ExitStack() as ctx, \
            nc.allow_low_precision(reason="fp32r rounding is intended"):
        consts = ctx.enter_context(tc.tile_pool(name="consts", bufs=1))
        big = ctx.enter_context(tc.tile_pool(name="big", bufs=1))
        raw = ctx.enter_context(tc.tile_pool(name="raw", bufs=1))
        qt_pool = ctx.enter_context(tc.tile_pool(name="qtp", bufs=1))
        et_pool = ctx.enter_context(tc.tile_pool(name="etp", bufs=4))
        small = ctx.enter_context(tc.tile_pool(name="small", bufs=4))
        ostage = ctx.enter_context(tc.tile_pool(name="ostage", bufs=4))
        ps_st = ctx.enter_context(tc.tile_pool(name="ps_st", bufs=4,
                                               space="PSUM"))
        ps_acc = ctx.enter_context(tc.tile_pool(name="ps_acc", bufs=1,
                                                space="PSUM"))

        ident = consts.tile([P, P], F32)
        make_identity(nc, ident)
        ident_bf = consts.tile([P, P], BF16)
        nc.vector.tensor_copy(ident_bf, ident)

        # K_aug / V_aug: [p, kc, 0:256] = row kc*P+p; col 256 = 1.0
        k_aug = big.tile([P, KC, DA], BF16)
        v_aug = big.tile([P, KC, DA], BF16)
        kt_sb = big.tile([P, DC, K], BF16)

        def load_aug(mat_dram, mat_aug):
            stage = raw.tile([P, KC, DA], F32, tag="raw")
            nc.sync.dma_start(stage[:, :, 0:D],
                              mat_dram.rearrange("(kc p) d -> p kc d", p=P))
            nc.vector.memset(stage[:, :, D:DA], 1.0)
            nc.vector.tensor_copy(mat_aug[:], stage[:])

        CH = 2  # k-chunks per load chunk
        for c0 in range(0, KC, CH):
            stage_c = raw.tile([P, CH, DA], F32, tag="rawc", bufs=6)
            nc.sync.dma_start(
                stage_c[:, :, 0:D],
                k_dram[c0 * P:(c0 + CH) * P, :].rearrange(
                    "(kc p) d -> p kc d", p=P))
            nc.vector.memset(stage_c[:, :, D:DA], 1.0)
            nc.vector.tensor_copy(k_aug[:, c0:c0 + CH, :], stage_c[:])
            for kc in range(c0, c0 + CH):
                for dc in range(DC):
                    tp = ps_st.tile([P, P], BF16, tag="st")
                    nc.tensor.transpose(
                        tp, k_aug[:, kc, dc * P:(dc + 1) * P], ident_bf)
                    nc.scalar.copy(kt_sb[:, dc, kc * P:(kc + 1) * P], tp)

        first_rep = True
        for _ in range(reps):
            # Q^T: qt[p, dc, t] = q[t, dc*P+p]; updated in place each step.
            q_raw = raw.tile([P, n_tc, D], F32, tag="raw")
            nc.sync.dma_start(q_raw[:],
                              q_dram.rearrange("(tc p) d -> p tc d", p=P))
            qts = [qt_pool.tile([P, DC, tt_sz], BF16, tag=f"qt{tt}",
                                name=f"qt{tt}")
                   for tt in range(t_tiles)]
            for tcn in range(n_tc):
                tt, off = divmod(tcn * P, tt_sz)
                for dc in range(DC):
                    tp = ps_st.tile([P, P], F32, tag="st")
                    nc.tensor.transpose(tp, q_raw[:, tcn, dc * P:(dc + 1) * P],
                                        ident)
                    nc.scalar.copy(qts[tt][:, dc, off:off + P], tp)
            if first_rep:
                first_rep = False
                load_aug(v_dram, v_aug)  # needed only in the last step

            pending = None  # lazily emitted epilogue of previous t-tile
            for step in range(NSTEPS):
                last = step == NSTEPS - 1
                mat = v_aug if last else k_aug

                segs = [(tt, 0, tt_sz) for tt in range(t_tiles)]
                if last and tt_sz >= 2 * P:
                    # split the final tile so the serial tail epilogue is
                    # half the size
                    tti, off, sz = segs[-1]
                    segs[-1:] = [(tti, off, sz // 2),
                                 (tti, off + sz // 2, sz - sz // 2)]
                for tti, off, sz in segs:
                    if pending is not None and pending[1] == tti:
                        # epilogue writes qt[tti]; must precede this tile's
                        # scores in program order
                        pending[0]()
                        pending = None
                    qt = qts[tti]
                    n_sb = sz // P
                    acc = ps_acc.tile([P, n_sb, DA], F32, tag="acc",
                                      padded_shape=[P, n_sub, 512])

                    def qupd(pet, pkc, acc=acc, mat=mat, n_sb=n_sb):
                        for j in range(n_sb):
                            nc.tensor.matmul(
                                acc[:, j, :],
                                pet[:, bass.ts(j, P)],
                                mat[:, pkc, :],
                                start=(pkc == 0), stop=(pkc == KC - 1))

                    prev = None
                    for kc in range(KC):
                        st = ps_st.tile([P, sz], F32, tag="st")
                        for dc in range(DC):
                            nc.tensor.matmul(
                                st,
                                kt_sb[:, dc, kc * P:(kc + 1) * P],
                                qt[:, dc, off:off + sz],
                                start=(dc == 0), stop=(dc == DC - 1))
                        et = et_pool.tile([P, sz], BF16, tag="et", bufs=10)
                        nc.scalar.activation(et, st, EXP, scale=beta)
                        if prev is not None:
                            qupd(*prev)
                        prev = (et, kc)
                        if kc == 1 and pending is not None:
                            # previous tile's epilogue hides under this loop
                            pending[0]()
                            pending = None
                    qupd(*prev)

                    def epilogue(acc=acc, tti=tti, off=off, n_sb=n_sb,
                                 qt=qt, last=last):
                        # single fast DVE copy releases the PSUM accumulator
                        acc_sb = small.tile([P, n_sb, DA], F32, tag="accs",
                                            bufs=4)
                        nc.vector.tensor_copy(acc_sb, acc)
                        for j in range(n_sb):
                            row0 = tti * tt_sz + off + j * P
                            qoff = off + j * P
                            rcp = small.tile([P, 1], F32, tag="rcp")
                            nc.vector.reciprocal(rcp, acc_sb[:, j, D:D + 1])
                            if not last:
                                qn = small.tile([P, D], F32, tag="qn")
                                nc.vector.tensor_scalar_mul(
                                    qn, acc_sb[:, j, 0:D], rcp)
                                tp = ps_st.tile([P, D], F32, tag="st")
                                for dc in range(DC):
                                    nc.tensor.transpose(
                                        tp[:, dc * P:(dc + 1) * P],
                                        qn[:, dc * P:(dc + 1) * P], ident)
                                for dc in range(DC):
                                    nc.vector.tensor_copy(
                                        qt[:, dc, qoff:qoff + P],
                                        tp[:, dc * P:(dc + 1) * P])
                            else:
                                ret_sb = ostage.tile([P, D], F32, tag="ret")
                                nc.vector.tensor_scalar_mul(
                                    ret_sb, acc_sb[:, j, 0:D], rcp)
                                nc.sync.dma_start(
                                    retr_dram[row0:row0 + P, :], ret_sb)
                                # attention rows, natural orientation
                                for kb in range(K // 512):
                                    s3n = ps_st.tile([P, 512], F32, tag="st")
                                    for dc in range(DC):
                                        nc.tensor.matmul(
                                            s3n,
                                            qt[:, dc, qoff:qoff + P],
                                            kt_sb[:, dc,
                                                  kb * 512:(kb + 1) * 512],
                                            start=(dc == 0),
                                            stop=(dc == DC - 1))
                                    a_sb = ostage.tile([P, 512], F32,
                                                       tag="attn", bufs=8)
                                    nc.scalar.activation(a_sb, s3n, EXP,
                                                         scale=beta)
                                    nc.vector.tensor_scalar_mul(a_sb, a_sb,
                                                                rcp)
                                    nc.sync.dma_start(
                                        attn_dram[row0:row0 + P,
                                                  kb * 512:(kb + 1) * 512],
                                        a_sb)

                    pending = (epilogue, tti)
            if pending is not None:
                pending[0]()

    nc.compile()
    return nc


def kernel(query, keys, values, log_beta):
    beta = float(np.exp(np.float32(log_beta)))
    nc = build_kernel(beta)
    qf = np.ascontiguousarray(
        np.asarray(query, dtype=np.float32).reshape(B * TFULL, D))
    keys = np.ascontiguousarray(np.asarray(keys, dtype=np.float32))
    values = np.ascontiguousarray(np.asarray(values, dtype=np.float32))
    in_maps = [
        {"query": qf[c * T:(c + 1) * T], "keys": keys, "values": values}
        for c in range(N_CORES)
    ]
    res = run_bass_kernel_spmd(nc, in_maps, core_ids=list(range(N_CORES)))
    retrieved = np.concatenate(
        [res.results[c]["retrieved"] for c in range(N_CORES)], axis=0)
    attn = np.concatenate(
        [res.results[c]["attn"] for c in range(N_CORES)], axis=0)
    return (retrieved.reshape(B, TFULL, D).astype(np.float32),
            attn.reshape(B, TFULL, K).astype(np.float32))


# revision 37
# speedup vs baseline: 1.3183x; 1.0018x over previous
"""Trainium2 Bass kernel for iterative Hopfield associative memory retrieval.

Reference computation (per problem):
    for 3 steps: scores = q @ K^T * beta; attn = softmax(scores); q = attn @ K
    retrieved = attn @ V
    returns (retrieved, attn)

Sharding: data-parallel over the flattened (batch*token) dim across 8 cores;
keys/values replicated. Each core handles 2048 tokens.

Per-core kernel strategy:
  - Scores are computed transposed (S^T [k, t]) so exp(S^T) tiles have the
    contraction dim k on partitions and feed the q-update / retrieval matmuls
    as stationary operands directly.
  - K and V are augmented with a ones column (K_aug [k, 257]) used as the
    moving operand: each accumulation produces the updated q (natural layout)
    PLUS the softmax row-sum in column 256 — no separate row-sum matmuls.
  - q-update: normalize by 1/rowsum (per-partition scalar), transpose back to
    [d, t] for the next step's scores.
  - Final step: the augmented-V accumulation yields retrieved (natural) +
    rowsums; attention is recomputed in natural [t, k] orientation, exp'd on
    the scalar engine (no table-switching functions), normalized by a
    per-partition 1/rowsum multiply on the vector engine, and DMA'd out.
  - Matmul operands are bf16 (same 1 cycle/row as fp32r but with fast
    weight loads), accumulation in fp32 PSUM; all outputs stay fp32.
    Measured end-to-end error ~2e-3 rel-L2 (gate 2e-2): the peaked softmax
    cancels correlated rounding errors.
  - Emission is software-pipelined: scores(kc+1) is emitted before the
    q-update of kc; per-tile epilogues are emitted after the next tile's
    main loop so the PE never waits on the normalize chain.
"""

from contextlib import ExitStack

import numpy as np

import concourse.bass as bass
import concourse.tile as tile
from concourse import bacc, mybir
from concourse.bass_utils import run_bass_kernel_spmd
from concourse.masks import make_identity

N_CORES = 8
B, TFULL, D = 2, 8192, 256
K = 4096
T = B * TFULL // N_CORES  # tokens per core = 2048
P = 128
DA = D + 2  # ones column (-> rowsum) + pad to even element count
TT = 512  # token tile
KC = K // P  # 32 key chunks
DC = D // P  # 2 dim chunks
NSTEPS = 3

F32 = mybir.dt.float32
BF16 = mybir.dt.bfloat16
FP8 = mybir.dt.float8e4
DR = mybir.MatmulPerfMode.DoubleRow
EXP = mybir.ActivationFunctionType.Exp


def build_kernel(beta: float, n_tok: int = T, reps: int = 1):
    t_tiles = max(1, n_tok // TT)
    tt_sz = min(TT, n_tok)
    n_sub = tt_sz // P
    n_tc = n_tok // P
    nc = bacc.Bacc("TRN2", target_bir_lowering=False, debug=False,
                   num_devices=N_CORES)
    q_dram = nc.dram_tensor("query", [n_tok, D], F32, kind="ExternalInput").ap()
    k_dram = nc.dram_tensor("keys", [K, D], F32, kind="ExternalInput").ap()
    v_dram = nc.dram_tensor("values", [K, D], F32, kind="ExternalInput").ap()
    retr_dram = nc.dram_tensor("retrieved", [n_tok, D], F32,
                               kind="ExternalOutput").ap()
    attn_dram = nc.dram_tensor("attn", [n_tok, K], F32,
                               kind="ExternalOutput").ap()

    with tile.TileContext(nc) as tc, ExitStack() as ctx, \
            nc.allow_low_precision(reason="fp32r rounding is intended"):
        consts = ctx.enter_context(tc.tile_pool(name="consts", bufs=1))
        big = ctx.enter_context(tc.tile_pool(name="big", bufs=1))
        raw = ctx.enter_context(tc.tile_pool(name="raw", bufs=1))
        qt_pool = ctx.enter_context(tc.tile_pool(name="qtp", bufs=1))
        et_pool = ctx.enter_context(tc.tile_pool(name="etp", bufs=4))
        small = ctx.enter_context(tc.tile_pool(name="small", bufs=4))
        ostage = ctx.enter_context(tc.tile_pool(name="ostage", bufs=4))
        ps_st = ctx.enter_context(tc.tile_pool(name="ps_st", bufs=4,
                                               space="PSUM"))
        ps_acc = ctx.enter_context(tc.tile_pool(name="ps_acc", bufs=1,
                                                space="PSUM"))

        ident = consts.tile([P, P], F32)
        make_identity(nc, ident)
        ident_bf = consts.tile([P, P], BF16)
        nc.vector.tensor_copy(ident_bf, ident)

        # K_aug / V_aug: [p, kc, 0:256] = row kc*P+p; col 256 = 1.0
        k_aug = big.tile([P, KC, DA], BF16)
        v_aug = big.tile([P, KC, DA], BF16)
        kt_sb = big.tile([P, DC, K], FP8)

        def load_aug(mat_dram, mat_aug):
            stage = raw.tile([P, KC, DA], F32, tag="raw")
            nc.sync.dma_start(stage[:, :, 0:D],
                              mat_dram.rearrange("(kc p) d -> p kc d", p=P))
            nc.vector.memset(stage[:, :, D:DA], 1.0)
            nc.vector.tensor_copy(mat_aug[:], stage[:])

        CH = 2  # k-chunks per load chunk
        for c0 in range(0, KC, CH):
            stage_c = raw.tile([P, CH, DA], F32, tag="rawc", bufs=6)
            nc.sync.dma_start(
                stage_c[:, :, 0:D],
                k_dram[c0 * P:(c0 + CH) * P, :].rearrange(
                    "(kc p) d -> p kc d", p=P))
            nc.vector.memset(stage_c[:, :, D:DA], 1.0)
            nc.vector.tensor_copy(k_aug[:, c0:c0 + CH, :], stage_c[:])
            for kc in range(c0, c0 + CH):
                for dc in range(DC):
                    tp = ps_st.tile([P, P], BF16, tag="st")
                    nc.tensor.transpose(
                        tp, k_aug[:, kc, dc * P:(dc + 1) * P], ident_bf)
                    nc.scalar.copy(kt_sb[:, dc, kc * P:(kc + 1) * P], tp)

        first_rep = True
        for _ in range(reps):
            # Q^T: qt[p, dc, t] = q[t, dc*P+p]; updated in place each step.
            q_raw = raw.tile([P, n_tc, D], F32, tag="raw")
            nc.sync.dma_start(q_raw[:],
                              q_dram.rearrange("(tc p) d -> p tc d", p=P))
            qts = [qt_pool.tile([P, DC, tt_sz], FP8, tag=f"qt{tt}",
                                name=f"qt{tt}")
                   for tt in range(t_tiles)]
            for tcn in range(n_tc):
                tt, off = divmod(tcn * P, tt_sz)
                for dc in range(DC):
                    tp = ps_st.tile([P, P], F32, tag="st")
                    nc.tensor.transpose(tp, q_raw[:, tcn, dc * P:(dc + 1) * P],
                                        ident)
                    nc.scalar.copy(qts[tt][:, dc, off:off + P], tp)
            if first_rep:
                first_rep = False
                load_aug(v_dram, v_aug)  # needed only in the last step

            pending = None  # lazily emitted epilogue of previous t-tile
            for step in range(NSTEPS):
                last = step == NSTEPS - 1
                mat = v_aug if last else k_aug

                segs = [(tt, 0, tt_sz) for tt in range(t_tiles)]
                if last and tt_sz >= 2 * P:
                    # split the final tile so the serial tail epilogue is
                    # half the size
                    tti, off, sz = segs[-1]
                    segs[-1:] = [(tti, off, sz // 2),
                                 (tti, off + sz // 2, sz - sz // 2)]
                for tti, off, sz in segs:
                    if pending is not None and pending[1] == tti:
                        # epilogue writes qt[tti]; must precede this tile's
                        # scores in program order
                        pending[0]()
                        pending = None
                    qt = qts[tti]
                    n_sb = sz // P
                    acc = ps_acc.tile([P, n_sb, DA], F32, tag="acc",
                                      padded_shape=[P, n_sub, 512])

                    def qupd(pet, pkc, acc=acc, mat=mat, n_sb=n_sb):
                        for j in range(n_sb):
                            nc.tensor.matmul(
                                acc[:, j, :],
                                pet[:, bass.ts(j, P)],
                                mat[:, pkc, :],
                                start=(pkc == 0), stop=(pkc == KC - 1))

                    prev = None
                    for kc in range(KC):
                        st = ps_st.tile([P, sz], F32, tag="st")
                        nc.tensor.matmul(
                            st,
                            kt_sb[:, :, kc * P:(kc + 1) * P],
                            qt[:, :, off:off + sz],
                            start=True, stop=True, perf_mode=DR)
                        et = et_pool.tile([P, sz], BF16, tag="et", bufs=10)
                        nc.scalar.activation(et, st, EXP, scale=beta)
                        if prev is not None:
                            qupd(*prev)
                        prev = (et, kc)
                        if kc == 1 and pending is not None:
                            # previous tile's epilogue hides under this loop
                            pending[0]()
                            pending = None
                    qupd(*prev)

                    def epilogue(acc=acc, tti=tti, off=off, n_sb=n_sb,
                                 qt=qt, last=last):
                        # single fast DVE copy releases the PSUM accumulator
                        acc_sb = small.tile([P, n_sb, DA], F32, tag="accs",
                                            bufs=4)
                        nc.vector.tensor_copy(acc_sb, acc)
                        for j in range(n_sb):
                            row0 = tti * tt_sz + off + j * P
                            qoff = off + j * P
                            rcp = small.tile([P, 1], F32, tag="rcp")
                            nc.vector.reciprocal(rcp, acc_sb[:, j, D:D + 1])
                            if not last:
                                qn = small.tile([P, D], F32, tag="qn")
                                nc.vector.tensor_scalar_mul(
                                    qn, acc_sb[:, j, 0:D], rcp)
                                tp = ps_st.tile([P, D], F32, tag="st")
                                for dc in range(DC):
                                    nc.tensor.transpose(
                                        tp[:, dc * P:(dc + 1) * P],
                                        qn[:, dc * P:(dc + 1) * P], ident)
                                for dc in range(DC):
                                    nc.vector.tensor_copy(
                                        qt[:, dc, qoff:qoff + P],
                                        tp[:, dc * P:(dc + 1) * P])
                            else:
                                ret_sb = ostage.tile([P, D], F32, tag="ret")
                                nc.vector.tensor_scalar_mul(
                                    ret_sb, acc_sb[:, j, 0:D], rcp)
                                nc.sync.dma_start(
                                    retr_dram[row0:row0 + P, :], ret_sb)
                                # attention rows, natural orientation
                                for kb in range(K // 512):
                                    s3n = ps_st.tile([P, 512], F32, tag="st")
                                    nc.tensor.matmul(
                                        s3n,
                                        qt[:, :, qoff:qoff + P],
                                        kt_sb[:, :,
                                              kb * 512:(kb + 1) * 512],
                                        start=True, stop=True, perf_mode=DR)
                                    a_sb = ostage.tile([P, 512], F32,
                                                       tag="attn", bufs=8)
                                    nc.scalar.activation(a_sb, s3n, EXP,
                                                         scale=beta)
                                    nc.vector.tensor_scalar_mul(a_sb, a_sb,
                                                                rcp)
                                    nc.sync.dma_start(
                                        attn_dram[row0:row0 + P,
                                                  kb * 512:(kb + 1) * 512],
                                        a_sb)

                    pending = (epilogue, tti)
            if pending is not None:
                pending[0]()

    nc.compile()
    return nc


def kernel(query, keys, values, log_beta):
    beta = float(np.exp(np.float32(log_beta)))
    nc = build_kernel(beta)
    qf = np.ascontiguousarray(
        np.asarray(query, dtype=np.float32).reshape(B * TFULL, D))
    keys = np.ascontiguousarray(np.asarray(keys, dtype=np.float32))
    values = np.ascontiguousarray(np.asarray(values, dtype=np.float32))
    in_maps = [
        {"query": qf[c * T:(c + 1) * T], "keys": keys, "values": values}
        for c in range(N_CORES)
    ]
    res = run_bass_kernel_spmd(nc, in_maps, core_ids=list(range(N_CORES)))
    retrieved = np.concatenate(
        [res.results[c]["retrieved"] for c in range(N_CORES)], axis=0)
    attn = np.concatenate(
        [res.results[c]["attn"] for c in range(N_CORES)], axis=0)
    return (retrieved.reshape(B, TFULL, D).astype(np.float32),
            attn.reshape(B, TFULL, K).astype(np.float32))
